# revision 6
# baseline (speedup 1.0000x reference)
"""Trainium2 Bass kernel for MultiHeadEdgeAttention_ParallelBetweenBondtypes.

Problem (B=16, N=256, H=8, d=64, T=5):
    s = src.reshape(B,N,H,d); A = a.reshape(H,d,d,T)
    scores[b,h,i,j,t] = sum_{d,e} s[b,i,h,d] A[h,d,e,t] dst[b,j,h,e]
    out = leaky_relu(where(edges==-1, -1e10, scores[...,edges]), 0.2)

Strategy (data-parallel over B across 8 cores, 2 batches/core):
  - Host: transpose src/dst to [B, H*d, N], cast edges to f32, pre-transpose
    the parameter a to per-(h,t) [e,d] blocks.
  - Per (b,h): W_t = A_t @ dstT  (5 matmuls, f32r, N=256) -> PSUM -> SBUF
  - Per (b,h,i-half): scores planes S_t = srcT_half.T @ W  (3 matmuls f32r,
    N=512/512/256) -> PSUM [128, (t,j)=1280]
  - Select: one-hot masks built per-b from edges (t==edges), product in
    bf16 on DVE, tree-sum over the 5 planes plus a -1e10 masked-edge plane,
    leaky-relu on ACT (Prelu alpha=0.2), DMA out.
"""

import sys
import types

for _p in ("/opt/trn_rl_repo", "/root/.axon_site/_ro/trn_rl_repo"):
    if _p not in sys.path:
        sys.path.append(_p)

import numpy as np

# ---------------------------------------------------------------------------
# Optional NTFF profiling hook (axon images lack antenv.axon_hooks; provide it
# so run_bass_kernel_spmd(trace=True) can capture HW exec time).
# ---------------------------------------------------------------------------
def _install_ntff_hook():
    if "antenv.axon_hooks" in sys.modules:
        return
    try:
        import antenv
        from trn_agent_boot.trn_boot import _ntff_profile_via_ctypes
    except Exception:
        return
    try:
        hook = _ntff_profile_via_ctypes("/opt/axon/libaxon_pjrt.so")
    except Exception:
        hook = None
    mod = types.ModuleType("antenv.axon_hooks")
    mod._hook = hook
    mod.get_axon_ntff_profile_hook = lambda: mod._hook
    def _set(h):
        mod._hook = h
    mod.set_axon_ntff_profile_hook = _set
    sys.modules["antenv.axon_hooks"] = mod
    antenv.axon_hooks = mod


_install_ntff_hook()

import bass_rust
import concourse.bass as bass
import concourse.mybir as mybir
import concourse.tile as tile
from concourse import bass_utils
from concourse.tile import ScopedClock

# avoid bucket uploads from the trace path in this sandbox
bass_utils.upload_artifacts = lambda tmpdir: ""

B, N, H, D, T = 16, 256, 8, 64, 5
N_CORES = 8
B_LOC = B // N_CORES
HALF = N // 2  # 128-row halves of the i axis
MASK_VAL = -1e10
NEG_SLOPE = 0.2

F32 = mybir.dt.float32
F32R = mybir.dt.float32r
BF16 = mybir.dt.bfloat16
ALU = mybir.AluOpType


def _patched_drain_and_barrier(self, tick_clock, wait_clock):
    """Tile tail drain with >1 sem wait breaks this walrus build
    (setupSyncWait: 'Too many sync wait commands'). Split the waits across
    multiple drain instructions, one wait each."""
    nc = self.nc
    drain_inst = nc.sync.drain()
    wait_clock.add_sem_waits(drain_inst.ins, ScopedClock({None: tick_clock.global_clock}))
    si = drain_inst.ins.sync_info
    waits = list(si.on_wait or []) if si is not None else []
    if len(waits) > 1:
        drain_inst.ins.sync_info = bass_rust.SyncInfo(
            on_wait=waits[:1], on_update=list(si.on_update or [])
        )
        rest = waits[1:]
        while rest:
            extra = nc.sync.drain()
            extra.ins.sync_info = bass_rust.SyncInfo(on_wait=rest[:1], on_update=[])
            rest = rest[1:]
    nc.all_engine_barrier()
    popped = nc._tile_sem_poison_stack.pop()
    assert popped is self._sem_poison
    nc.clear_and_free_semaphores(list(self.sems.allocated().values()))
    nc.all_engine_barrier()


tile.TileContext._drain_and_barrier = _patched_drain_and_barrier


def _split_waits(nc, max_waits=1):
    """This walrus build rejects instructions carrying more than one sem wait
    ('Too many sync wait commands', e.g. on fused-matmul LDWEIGHTS and CTRL
    drain templates). Hoist extra waits onto same-engine NoOps inserted
    immediately before the over-waited instruction (engine execution is
    in-order, so semantics are preserved)."""
    for f in nc.m.functions:
        for bb in f.blocks:
            insts = bb.instructions
            if not any(
                inst.sync_info and inst.sync_info.on_wait
                and len(inst.sync_info.on_wait) > max_waits
                for inst in insts
            ):
                continue
            new_list = []
            for inst in insts:
                si = inst.sync_info
                waits = list(si.on_wait or []) if si is not None else []
                if len(waits) > max_waits:
                    for w in waits[:-max_waits]:
                        nop = bass_rust.InstNoOp(
                            name=nc.get_next_instruction_name(), engine=inst.engine
                        )
                        nop.sync_info = bass_rust.SyncInfo(on_wait=[w], on_update=[])
                        new_list.append(nop)
                    inst.sync_info = bass_rust.SyncInfo(
                        on_wait=waits[-max_waits:], on_update=list(si.on_update or [])
                    )
                new_list.append(inst)
            insts[:] = new_list


def _build_program():
    nc = bass.Bass("TRN2", target_bir_lowering=False, debug=False)

    # host-prepared layouts (see _prep_in_maps):
    #   srcT: [B_LOC, d, (h,i)]   dstT: [B_LOC, e, (h,j)]
    #   at:   [e, (h,t,d)]        edgf: [B_LOC, p, (g,j)]  (i = g*128+p)
    srcT = nc.dram_tensor("srcT", [B_LOC, D, H * N], F32R, kind="ExternalInput").ap()
    dstT = nc.dram_tensor("dstT", [B_LOC, D, H * N], F32R, kind="ExternalInput").ap()
    at = nc.dram_tensor("at", [D, H * T * D], F32R, kind="ExternalInput").ap()
    edgf = nc.dram_tensor("edgf", [B_LOC, HALF, 2 * N], F32, kind="ExternalInput").ap()
    out = nc.dram_tensor("out", [B_LOC, H, N, N], F32, kind="ExternalOutput").ap()

    with tile.TileContext(nc) as tc:
        with (
            tc.tile_pool(name="const", bufs=1) as const_pool,
            tc.tile_pool(name="perb", bufs=2) as perb_pool,
            tc.tile_pool(name="perh", bufs=2) as perh_pool,
            tc.tile_pool(name="perg", bufs=3) as perg_pool,
            tc.tile_pool(name="wps", bufs=1, space="PSUM") as wps_pool,
            tc.tile_pool(name="sps", bufs=2, space="PSUM") as sps_pool,
            tc.tile_pool(name="sbps", bufs=1, space="PSUM") as sbps_pool,
        ):
            # parameter blocks: at[h,t] is [e,d]; lay out as [e=64, (h,t,d)]
            at_sb = const_pool.tile([D, H * T * D], F32R)
            nc.sync.dma_start(out=at_sb[:], in_=at[:])

            for b in range(B_LOC):
                srcT_sb = perb_pool.tile([D, H * N], F32R, tag="srcT")
                nc.sync.dma_start(out=srcT_sb[:], in_=srcT[b])
                dstT_sb = perb_pool.tile([D, H * N], F32R, tag="dstT")
                nc.sync.dma_start(out=dstT_sb[:], in_=dstT[b])
                edg_sb = perb_pool.tile([HALF, 2 * N], F32, tag="edg")
                nc.sync.dma_start(out=edg_sb[:], in_=edgf[b])

                # one-hot planes, bf16: oh[:, (g,t,j)] = (edges == t)
                oh_sb = perb_pool.tile([HALF, 2 * T * N], BF16, tag="oh")
                for g in range(2):
                    for t in range(T):
                        nc.vector.tensor_scalar(
                            out=oh_sb[:, (g * T + t) * N : (g * T + t + 1) * N],
                            in0=edg_sb[:, g * N : (g + 1) * N],
                            scalar1=float(t),
                            scalar2=None,
                            op0=ALU.is_equal,
                        )
                # masked-edge plane: (edges == -1) * MASK_VAL
                mn_sb = perb_pool.tile([HALF, 2 * N], BF16, tag="mn")
                nc.vector.tensor_scalar(
                    out=mn_sb[:],
                    in0=edg_sb[:],
                    scalar1=-1.0,
                    scalar2=MASK_VAL,
                    op0=ALU.is_equal,
                    op1=ALU.mult,
                )

                for h in range(H):
                    # ---- W_t = A_t @ dstT : [d=64, (t,j)=1280] ----
                    wA = wps_pool.tile([D, 4 * N], F32, tag="wA")
                    wB = wps_pool.tile([D, N], F32, tag="wB")
                    for t in range(T):
                        dst_ap = wA[:, t * N : (t + 1) * N] if t < 4 else wB[:]
                        nc.tensor.matmul(
                            dst_ap,
                            lhsT=at_sb[:, (h * T + t) * D : (h * T + t + 1) * D],
                            rhs=dstT_sb[:, h * N : (h + 1) * N],
                            start=True,
                            stop=True,
                        )
                    w_sb = perh_pool.tile([D, T * N], F32R, tag="w")
                    nc.scalar.copy(w_sb[:, : 4 * N], wA[:])
                    nc.scalar.copy(w_sb[:, 4 * N :], wB[:])

                    for g in range(2):
                        # ---- scores planes: [i=128, (t,j)=1280] ----
                        sA = sps_pool.tile([HALF, 2 * N], F32, tag="sA")
                        sB = sbps_pool.tile([HALF, N], F32, tag="sB")
                        lhsT = srcT_sb[:, h * N + g * HALF : h * N + (g + 1) * HALF]
                        nc.tensor.matmul(
                            sA[:, : 2 * N], lhsT=lhsT, rhs=w_sb[:, : 2 * N],
                            start=True, stop=True,
                        )
                        sA2 = sps_pool.tile([HALF, 2 * N], F32, tag="sA2")
                        nc.tensor.matmul(
                            sA2[:, : 2 * N], lhsT=lhsT, rhs=w_sb[:, 2 * N : 4 * N],
                            start=True, stop=True,
                        )
                        nc.tensor.matmul(
                            sB[:], lhsT=lhsT, rhs=w_sb[:, 4 * N :],
                            start=True, stop=True,
                        )

                        # ---- select: prod = S_t * onehot_t (bf16) ----
                        oh_g = oh_sb[:, g * T * N : (g + 1) * T * N]
                        prodA = perg_pool.tile([HALF, 2 * N], BF16, tag="prodA")
                        nc.vector.tensor_tensor(
                            out=prodA[:], in0=sA[:], in1=oh_g[:, : 2 * N],
                            op=ALU.mult,
                        )
                        prodB = perg_pool.tile([HALF, 2 * N], BF16, tag="prodB")
                        nc.vector.tensor_tensor(
                            out=prodB[:], in0=sA2[:], in1=oh_g[:, 2 * N : 4 * N],
                            op=ALU.mult,
                        )
                        prodC = perg_pool.tile([HALF, N], BF16, tag="prodC")
                        nc.vector.tensor_tensor(
                            out=prodC[:], in0=sB[:], in1=oh_g[:, 4 * N :],
                            op=ALU.mult,
                        )
                        # tree sum: (t0+t2, t1+t3) -> +(t4, mn) -> fold
                        r1 = perg_pool.tile([HALF, 2 * N], BF16, tag="r1")
                        nc.vector.tensor_tensor(
                            out=r1[:], in0=prodA[:], in1=prodB[:], op=ALU.add
                        )
                        r2 = perg_pool.tile([HALF, N], BF16, tag="r2")
                        nc.vector.tensor_tensor(
                            out=r2[:], in0=r1[:, :N], in1=r1[:, N:], op=ALU.add
                        )
                        r3 = perg_pool.tile([HALF, N], BF16, tag="r3")
                        nc.vector.tensor_tensor(
                            out=r3[:], in0=r2[:], in1=prodC[:], op=ALU.add
                        )
                        r4 = perg_pool.tile([HALF, N], BF16, tag="r4")
                        nc.vector.tensor_tensor(
                            out=r4[:], in0=r3[:], in1=mn_sb[:, g * N : (g + 1) * N],
                            op=ALU.add,
                        )
                        ob = perg_pool.tile([HALF, N], F32, tag="ob")
                        nc.scalar.activation(
                            ob[:], r4[:], mybir.ActivationFunctionType.Prelu,
                            bias=0.0, scale=1.0, alpha=NEG_SLOPE,
                        )
                        nc.sync.dma_start(
                            out=out[b, h, g * HALF : (g + 1) * HALF, :], in_=ob[:]
                        )
    _split_waits(nc)
    return nc


_STATE = {}


def _get_nc():
    if "nc" not in _STATE:
        _STATE["nc"] = _build_program()
    return _STATE["nc"]


def _prep_in_maps(src_embeddings, dst_embeddings, a, edges):
    src = np.asarray(src_embeddings, dtype=np.float32)
    dst = np.asarray(dst_embeddings, dtype=np.float32)
    a = np.asarray(a, dtype=np.float32)
    # [B, N, (h d)] -> [B, d, (h i)]
    srcT = np.ascontiguousarray(
        src.reshape(B, N, H, D).transpose(0, 3, 2, 1)
    ).reshape(B, D, H * N)
    dstT = np.ascontiguousarray(
        dst.reshape(B, N, H, D).transpose(0, 3, 2, 1)
    ).reshape(B, D, H * N)
    # a: [(h d), e, t] -> at[e, (h t d)]
    at = np.ascontiguousarray(
        a.reshape(H, D, D, T).transpose(2, 0, 3, 1)
    ).reshape(D, H * T * D)
    # edges: [B, (g p), j] -> [B, p, (g j)]
    edgf = np.ascontiguousarray(
        edges.astype(np.float32).reshape(B, 2, HALF, N).transpose(0, 2, 1, 3)
    ).reshape(B, HALF, 2 * N)
    in_maps = []
    for c in range(N_CORES):
        lo, hi = c * B_LOC, (c + 1) * B_LOC
        in_maps.append(
            {
                "srcT": srcT[lo:hi],
                "dstT": dstT[lo:hi],
                "at": at,
                "edgf": edgf[lo:hi],
            }
        )
    return in_maps


def run(src_embeddings, dst_embeddings, a, edges, trace=False):
    """Run on 8 cores; returns (output [B,H,N,N] f32, BassKernelResults)."""
    nc = _get_nc()
    in_maps = _prep_in_maps(src_embeddings, dst_embeddings, a, edges)
    res = bass_utils.run_bass_kernel_spmd(
        nc, in_maps, core_ids=list(range(N_CORES)), trace=trace
    )
    out = np.concatenate([res.results[c]["out"] for c in range(N_CORES)], axis=0)
    return out, res


def kernel(src_embeddings, dst_embeddings, a, edges):
    out, _ = run(src_embeddings, dst_embeddings, a, edges, trace=False)
    return out


# revision 12
# speedup vs baseline: 1.1831x; 1.1831x over previous
"""Trainium2 Bass kernel for MultiHeadEdgeAttention_ParallelBetweenBondtypes.

Problem (B=16, N=256, H=8, d=64, T=5):
    s = src.reshape(B,N,H,d); A = a.reshape(H,d,d,T)
    scores[b,h,i,j,t] = sum_{d,e} s[b,i,h,d] A[h,d,e,t] dst[b,j,h,e]
    out = leaky_relu(where(edges==-1, -1e10, scores[...,edges]), 0.2)

Strategy (data-parallel over B across 8 cores, 2 batches/core):
  - Host: transpose src/dst to [B, H*d, N], cast edges to f32, pre-transpose
    the parameter a to per-(h,t) [e,d] blocks.
  - Per (b,h): W_t = A_t @ dstT  (5 matmuls, f32r, N=256) -> PSUM -> SBUF
  - Per (b,h,i-half): scores planes S_t = srcT_half.T @ W  (3 matmuls f32r,
    N=512/512/256) -> PSUM [128, (t,j)=1280]
  - Select: one-hot masks built per-b from edges (t==edges), product in
    bf16 on DVE, tree-sum over the 5 planes plus a -1e10 masked-edge plane,
    leaky-relu on ACT (Prelu alpha=0.2), DMA out.
"""

import sys
import types

for _p in ("/opt/trn_rl_repo", "/root/.axon_site/_ro/trn_rl_repo"):
    if _p not in sys.path:
        sys.path.append(_p)

import numpy as np

# ---------------------------------------------------------------------------
# Optional NTFF profiling hook (axon images lack antenv.axon_hooks; provide it
# so run_bass_kernel_spmd(trace=True) can capture HW exec time).
# ---------------------------------------------------------------------------
def _install_ntff_hook():
    if "antenv.axon_hooks" in sys.modules:
        return
    try:
        import antenv
        from trn_agent_boot.trn_boot import _ntff_profile_via_ctypes
    except Exception:
        return
    try:
        hook = _ntff_profile_via_ctypes("/opt/axon/libaxon_pjrt.so")
    except Exception:
        hook = None
    mod = types.ModuleType("antenv.axon_hooks")
    mod._hook = hook
    mod.get_axon_ntff_profile_hook = lambda: mod._hook
    def _set(h):
        mod._hook = h
    mod.set_axon_ntff_profile_hook = _set
    sys.modules["antenv.axon_hooks"] = mod
    antenv.axon_hooks = mod


_install_ntff_hook()

import bass_rust
import concourse.bass as bass
import concourse.mybir as mybir
import concourse.tile as tile
from concourse import bass_utils
from concourse.tile import ScopedClock

# avoid bucket uploads from the trace path in this sandbox
bass_utils.upload_artifacts = lambda tmpdir: ""

B, N, H, D, T = 16, 256, 8, 64, 5
N_CORES = 8
B_LOC = B // N_CORES
HALF = N // 2  # 128-row halves of the i axis
MASK_VAL = -1e10
NEG_SLOPE = 0.2

F32 = mybir.dt.float32
F32R = mybir.dt.float32r
BF16 = mybir.dt.bfloat16
I16 = mybir.dt.int16
ALU = mybir.AluOpType


def _patched_drain_and_barrier(self, tick_clock, wait_clock):
    """Tile tail drain with >1 sem wait breaks this walrus build
    (setupSyncWait: 'Too many sync wait commands'). Split the waits across
    multiple drain instructions, one wait each."""
    nc = self.nc
    drain_inst = nc.sync.drain()
    wait_clock.add_sem_waits(drain_inst.ins, ScopedClock({None: tick_clock.global_clock}))
    si = drain_inst.ins.sync_info
    waits = list(si.on_wait or []) if si is not None else []
    if len(waits) > 1:
        drain_inst.ins.sync_info = bass_rust.SyncInfo(
            on_wait=waits[:1], on_update=list(si.on_update or [])
        )
        rest = waits[1:]
        while rest:
            extra = nc.sync.drain()
            extra.ins.sync_info = bass_rust.SyncInfo(on_wait=rest[:1], on_update=[])
            rest = rest[1:]
    nc.all_engine_barrier()
    popped = nc._tile_sem_poison_stack.pop()
    assert popped is self._sem_poison
    nc.clear_and_free_semaphores(list(self.sems.allocated().values()))
    nc.all_engine_barrier()


tile.TileContext._drain_and_barrier = _patched_drain_and_barrier


def _split_waits(nc, max_waits=1):
    """This walrus build rejects instructions carrying more than one sem wait
    ('Too many sync wait commands', e.g. on fused-matmul LDWEIGHTS and CTRL
    drain templates). Hoist extra waits onto same-engine NoOps inserted
    immediately before the over-waited instruction (engine execution is
    in-order, so semantics are preserved)."""
    for f in nc.m.functions:
        for bb in f.blocks:
            insts = bb.instructions
            if not any(
                inst.sync_info and inst.sync_info.on_wait
                and len(inst.sync_info.on_wait) > max_waits
                for inst in insts
            ):
                continue
            new_list = []
            for inst in insts:
                si = inst.sync_info
                waits = list(si.on_wait or []) if si is not None else []
                if len(waits) > max_waits:
                    for w in waits[:-max_waits]:
                        nop = bass_rust.InstNoOp(
                            name=nc.get_next_instruction_name(), engine=inst.engine
                        )
                        nop.sync_info = bass_rust.SyncInfo(on_wait=[w], on_update=[])
                        new_list.append(nop)
                    inst.sync_info = bass_rust.SyncInfo(
                        on_wait=waits[-max_waits:], on_update=list(si.on_update or [])
                    )
                new_list.append(inst)
            insts[:] = new_list


def _build_program():
    nc = bass.Bass("TRN2", target_bir_lowering=False, debug=False)

    # host-prepared layouts (see _prep_in_maps):
    #   srcT: [B_LOC, d, (h,i)]   dstT: [B_LOC, e, (h,j)]
    #   at:   [e, (h,t,d)]        edgb: [B_LOC, p, (g,j)] bf16  (i = g*128+p)
    srcT = nc.dram_tensor("srcT", [B_LOC, D, H * N], F32R, kind="ExternalInput").ap()
    dstT = nc.dram_tensor("dstT", [B_LOC, D, H * N], F32R, kind="ExternalInput").ap()
    at = nc.dram_tensor("at", [D, H * T * D], F32R, kind="ExternalInput").ap()
    edgb = nc.dram_tensor("edgb", [B_LOC, HALF, 2 * N], BF16, kind="ExternalInput").ap()
    out = nc.dram_tensor("out", [B_LOC, H, N, N], F32, kind="ExternalOutput").ap()

    with tile.TileContext(nc) as tc:
        with (
            tc.tile_pool(name="const", bufs=1) as const_pool,
            tc.tile_pool(name="perb", bufs=2) as perb_pool,
            tc.tile_pool(name="perh", bufs=2) as perh_pool,
            tc.tile_pool(name="perg", bufs=3) as perg_pool,
            tc.tile_pool(name="wps", bufs=2, space="PSUM") as wps_pool,
            tc.tile_pool(name="sps", bufs=2, space="PSUM") as sps_pool,
        ):
            # parameter blocks: at[h,t] is [e,d]; lay out as [e=64, (h,t,d)]
            at_sb = const_pool.tile([D, H * T * D], F32R)
            nc.sync.dma_start(out=at_sb[:], in_=at[:])

            for b in range(B_LOC):
                srcT_sb = perb_pool.tile([D, H * N], F32R, tag="srcT")
                nc.sync.dma_start(out=srcT_sb[:], in_=srcT[b])
                dstT_sb = perb_pool.tile([D, H * N], F32R, tag="dstT")
                nc.sync.dma_start(out=dstT_sb[:], in_=dstT[b])
                edg_sb = perb_pool.tile([HALF, 2 * N], BF16, tag="edg")
                nc.sync.dma_start(out=edg_sb[:], in_=edgb[b])

                # one-hot planes, bf16: oh[:, (t,g,j)] = (edges == t)
                # (t-major so each build op covers both halves at 4x mode)
                oh_sb = perb_pool.tile([HALF, T * 2 * N], I16, tag="oh")
                for t in range(T):
                    nc.vector.tensor_scalar(
                        out=oh_sb[:, t * 2 * N : (t + 1) * 2 * N],
                        in0=edg_sb[:],
                        scalar1=float(t),
                        scalar2=None,
                        op0=ALU.is_equal,
                    )
                # masked-edge init plane: (edges == -1) * MASK_VAL, exact fp32
                mn_sb = perb_pool.tile([HALF, 2 * N], F32, tag="mn")
                nc.vector.tensor_scalar(
                    out=mn_sb[:],
                    in0=edg_sb[:],
                    scalar1=-1.0,
                    scalar2=MASK_VAL,
                    op0=ALU.is_equal,
                    op1=ALU.mult,
                )

                for h in range(H):
                    # ---- W_t = A_t @ dstT : planes [d=64, j=256], t=0..4 ----
                    # pairs (t0,t1), (t2,t3), (t4) through [64,512] psum slots
                    w_sb = perh_pool.tile([D, T * N], F32R, tag="w")
                    for grp, (t0, nt) in enumerate(((0, 2), (2, 2), (4, 1))):
                        wp = wps_pool.tile([D, 2 * N], F32, tag="wp")
                        for k in range(nt):
                            t = t0 + k
                            nc.tensor.matmul(
                                wp[:, k * N : (k + 1) * N],
                                lhsT=at_sb[:, (h * T + t) * D : (h * T + t + 1) * D],
                                rhs=dstT_sb[:, h * N : (h + 1) * N],
                                start=True,
                                stop=True,
                            )
                        nc.scalar.copy(
                            w_sb[:, t0 * N : (t0 + nt) * N], wp[:, : nt * N]
                        )

                    for g in range(2):
                        # ---- scores planes: one psum tile [i=128, (t,j)=1280]
                        s_all = sps_pool.tile([HALF, T * N], F32, tag="s")
                        lhsT = srcT_sb[:, h * N + g * HALF : h * N + (g + 1) * HALF]
                        nc.tensor.matmul(
                            s_all[:, : 2 * N], lhsT=lhsT, rhs=w_sb[:, : 2 * N],
                            start=True, stop=True,
                        )
                        nc.tensor.matmul(
                            s_all[:, 2 * N : 4 * N], lhsT=lhsT,
                            rhs=w_sb[:, 2 * N : 4 * N], start=True, stop=True,
                        )
                        nc.tensor.matmul(
                            s_all[:, 4 * N :], lhsT=lhsT, rhs=w_sb[:, 4 * N :],
                            start=True, stop=True,
                        )

                        # ---- select: acc = mn; acc[oh_t] = S_t  (one pass) ----
                        acc = perg_pool.tile([HALF, N], F32, tag="acc")
                        nc.vector.tensor_copy(
                            out=acc[:], in_=mn_sb[:, g * N : (g + 1) * N]
                        )
                        # out is acc broadcast over t (stride-0); masks are
                        # disjoint so the overwrite order is irrelevant.
                        acc_b = acc[:, None, :].broadcast_to((HALF, T, N))
                        mask = oh_sb[:].rearrange("p (t gj) -> p t gj", t=T)[
                            :, :, g * N : (g + 1) * N
                        ]
                        data = s_all[:].rearrange("p (t j) -> p t j", t=T)
                        nc.vector.copy_predicated(acc_b, mask, data)

                        ob = perg_pool.tile([HALF, N], F32, tag="ob")
                        nc.scalar.activation(
                            ob[:], acc[:], mybir.ActivationFunctionType.Prelu,
                            bias=0.0, scale=1.0, alpha=NEG_SLOPE,
                        )
                        nc.sync.dma_start(
                            out=out[b, h, g * HALF : (g + 1) * HALF, :], in_=ob[:]
                        )
    _split_waits(nc)
    return nc


_STATE = {}


def _get_nc():
    if "nc" not in _STATE:
        _STATE["nc"] = _build_program()
    return _STATE["nc"]


def _prep_in_maps(src_embeddings, dst_embeddings, a, edges):
    src = np.asarray(src_embeddings, dtype=np.float32)
    dst = np.asarray(dst_embeddings, dtype=np.float32)
    a = np.asarray(a, dtype=np.float32)
    # [B, N, (h d)] -> [B, d, (h i)]
    srcT = np.ascontiguousarray(
        src.reshape(B, N, H, D).transpose(0, 3, 2, 1)
    ).reshape(B, D, H * N)
    dstT = np.ascontiguousarray(
        dst.reshape(B, N, H, D).transpose(0, 3, 2, 1)
    ).reshape(B, D, H * N)
    # a: [(h d), e, t] -> at[e, (h t d)]
    at = np.ascontiguousarray(
        a.reshape(H, D, D, T).transpose(2, 0, 3, 1)
    ).reshape(D, H * T * D)
    # edges: [B, (g p), j] -> [B, p, (g j)], as bf16 (values -1..4 are exact)
    import ml_dtypes

    edgb = np.ascontiguousarray(
        edges.astype(np.float32).reshape(B, 2, HALF, N).transpose(0, 2, 1, 3)
    ).reshape(B, HALF, 2 * N).astype(ml_dtypes.bfloat16)
    in_maps = []
    for c in range(N_CORES):
        lo, hi = c * B_LOC, (c + 1) * B_LOC
        in_maps.append(
            {
                "srcT": srcT[lo:hi],
                "dstT": dstT[lo:hi],
                "at": at,
                "edgb": edgb[lo:hi],
            }
        )
    return in_maps


def run(src_embeddings, dst_embeddings, a, edges, trace=False):
    """Run on 8 cores; returns (output [B,H,N,N] f32, BassKernelResults)."""
    nc = _get_nc()
    in_maps = _prep_in_maps(src_embeddings, dst_embeddings, a, edges)
    res = bass_utils.run_bass_kernel_spmd(
        nc, in_maps, core_ids=list(range(N_CORES)), trace=trace
    )
    out = np.concatenate([res.results[c]["out"] for c in range(N_CORES)], axis=0)
    return out, res


def kernel(src_embeddings, dst_embeddings, a, edges):
    out, _ = run(src_embeddings, dst_embeddings, a, edges, trace=False)
    return out


# revision 16
# speedup vs baseline: 1.2718x; 1.0750x over previous
"""Trainium2 Bass kernel for MultiHeadEdgeAttention_ParallelBetweenBondtypes.

Problem (B=16, N=256, H=8, d=64, T=5):
    s = src.reshape(B,N,H,d); A = a.reshape(H,d,d,T)
    scores[b,h,i,j,t] = sum_{d,e} s[b,i,h,d] A[h,d,e,t] dst[b,j,h,e]
    out = leaky_relu(where(edges==-1, -1e10, scores[...,edges]), 0.2)

Strategy (data-parallel over B across 8 cores, 2 batches/core):
  - Host: transpose src/dst to [B, H*d, N], cast edges to f32, pre-transpose
    the parameter a to per-(h,t) [e,d] blocks.
  - Per (b,h): W_t = A_t @ dstT  (5 matmuls, f32r, N=256) -> PSUM -> SBUF
  - Per (b,h,i-half): scores planes S_t = srcT_half.T @ W  (3 matmuls f32r,
    N=512/512/256) -> PSUM [128, (t,j)=1280]
  - Select: one-hot masks built per-b from edges (t==edges), product in
    bf16 on DVE, tree-sum over the 5 planes plus a -1e10 masked-edge plane,
    leaky-relu on ACT (Prelu alpha=0.2), DMA out.
"""

import sys
import types

for _p in ("/opt/trn_rl_repo", "/root/.axon_site/_ro/trn_rl_repo"):
    if _p not in sys.path:
        sys.path.append(_p)

import numpy as np

# ---------------------------------------------------------------------------
# Optional NTFF profiling hook (axon images lack antenv.axon_hooks; provide it
# so run_bass_kernel_spmd(trace=True) can capture HW exec time).
# ---------------------------------------------------------------------------
def _install_ntff_hook():
    if "antenv.axon_hooks" in sys.modules:
        return
    try:
        import antenv
        from trn_agent_boot.trn_boot import _ntff_profile_via_ctypes
    except Exception:
        return
    try:
        hook = _ntff_profile_via_ctypes("/opt/axon/libaxon_pjrt.so")
    except Exception:
        hook = None
    mod = types.ModuleType("antenv.axon_hooks")
    mod._hook = hook
    mod.get_axon_ntff_profile_hook = lambda: mod._hook
    def _set(h):
        mod._hook = h
    mod.set_axon_ntff_profile_hook = _set
    sys.modules["antenv.axon_hooks"] = mod
    antenv.axon_hooks = mod


_install_ntff_hook()

import bass_rust
import concourse.bass as bass
import concourse.mybir as mybir
import concourse.tile as tile
from concourse import bass_utils
from concourse.tile import ScopedClock

# avoid bucket uploads from the trace path in this sandbox
bass_utils.upload_artifacts = lambda tmpdir: ""

B, N, H, D, T = 16, 256, 8, 64, 5
N_CORES = 8
B_LOC = B // N_CORES
HALF = N // 2  # 128-row halves of the i axis
MASK_VAL = -1e10
NEG_SLOPE = 0.2

F32 = mybir.dt.float32
F32R = mybir.dt.float32r
BF16 = mybir.dt.bfloat16
I16 = mybir.dt.int16
ALU = mybir.AluOpType


def _patched_drain_and_barrier(self, tick_clock, wait_clock):
    """Tile tail drain with >1 sem wait breaks this walrus build
    (setupSyncWait: 'Too many sync wait commands'). Split the waits across
    multiple drain instructions, one wait each."""
    nc = self.nc
    drain_inst = nc.sync.drain()
    wait_clock.add_sem_waits(drain_inst.ins, ScopedClock({None: tick_clock.global_clock}))
    si = drain_inst.ins.sync_info
    waits = list(si.on_wait or []) if si is not None else []
    if len(waits) > 1:
        drain_inst.ins.sync_info = bass_rust.SyncInfo(
            on_wait=waits[:1], on_update=list(si.on_update or [])
        )
        rest = waits[1:]
        while rest:
            extra = nc.sync.drain()
            extra.ins.sync_info = bass_rust.SyncInfo(on_wait=rest[:1], on_update=[])
            rest = rest[1:]
    nc.all_engine_barrier()
    popped = nc._tile_sem_poison_stack.pop()
    assert popped is self._sem_poison
    nc.clear_and_free_semaphores(list(self.sems.allocated().values()))
    nc.all_engine_barrier()


tile.TileContext._drain_and_barrier = _patched_drain_and_barrier


def _split_waits(nc):
    """This walrus build rejects instructions carrying more than one sem wait
    ('Too many sync wait commands': S3_LW matmul, S3S3D3_TT tensor-tensor and
    CTRL templates were all observed to fail with 2). Hoist extra waits onto
    same-engine NoOps inserted immediately before the over-waited instruction
    (engine execution is in-order, so semantics are preserved)."""
    def cap(inst):
        return 1

    for f in nc.m.functions:
        for bb in f.blocks:
            insts = bb.instructions
            if not any(
                inst.sync_info and inst.sync_info.on_wait
                and len(inst.sync_info.on_wait) > cap(inst)
                for inst in insts
            ):
                continue
            new_list = []
            for inst in insts:
                si = inst.sync_info
                waits = list(si.on_wait or []) if si is not None else []
                mw = cap(inst)
                if len(waits) > mw:
                    for w in waits[:-mw]:
                        nop = bass_rust.InstNoOp(
                            name=nc.get_next_instruction_name(), engine=inst.engine
                        )
                        nop.sync_info = bass_rust.SyncInfo(on_wait=[w], on_update=[])
                        new_list.append(nop)
                    inst.sync_info = bass_rust.SyncInfo(
                        on_wait=waits[-mw:], on_update=list(si.on_update or [])
                    )
                new_list.append(inst)
            insts[:] = new_list


def _build_program():
    nc = bass.Bass("TRN2", target_bir_lowering=False, debug=False)

    # host-prepared layouts (see _prep_in_maps):
    #   srcT: [B_LOC, d, (h,i)]   dstT: [B_LOC, e, (h,j)]
    #   at:   [e, (h,t,d)]        edgb: [B_LOC, p, (g,j)] bf16  (i = g*128+p)
    srcT = nc.dram_tensor("srcT", [B_LOC, D, H * N], F32R, kind="ExternalInput").ap()
    dstT = nc.dram_tensor("dstT", [B_LOC, D, H * N], F32R, kind="ExternalInput").ap()
    at = nc.dram_tensor("at", [D, H * T * D], F32R, kind="ExternalInput").ap()
    edgb = nc.dram_tensor("edgb", [B_LOC, HALF, 2 * N], BF16, kind="ExternalInput").ap()
    out = nc.dram_tensor("out", [B_LOC, H, N, N], F32, kind="ExternalOutput").ap()

    with tile.TileContext(nc) as tc:
        with (
            tc.tile_pool(name="const", bufs=1) as const_pool,
            tc.tile_pool(name="perb", bufs=2) as perb_pool,
            tc.tile_pool(name="perh", bufs=2) as perh_pool,
            tc.tile_pool(name="perg", bufs=3) as perg_pool,
            tc.tile_pool(name="wps", bufs=2, space="PSUM") as wps_pool,
            tc.tile_pool(name="sps", bufs=2, space="PSUM") as sps_pool,
        ):
            # parameter blocks: at[h,t] is [e,d]; lay out as [e=64, (h,t,d)]
            at_sb = const_pool.tile([D, H * T * D], F32R)
            nc.sync.dma_start(out=at_sb[:], in_=at[:])

            for b in range(B_LOC):
                # edges first (one-hot build can start immediately), then
                # src/dst in per-2-head chunks so h=0 matmuls start early
                edg_sb = perb_pool.tile([HALF, 2 * N], BF16, tag="edg")
                nc.sync.dma_start(out=edg_sb[:], in_=edgb[b])
                srcT_sb = perb_pool.tile([D, H * N], F32R, tag="srcT")
                dstT_sb = perb_pool.tile([D, H * N], F32R, tag="dstT")
                for c0 in range(0, H, 2):
                    sl = slice(c0 * N, (c0 + 2) * N)
                    nc.sync.dma_start(out=dstT_sb[:, sl], in_=dstT[b][:, sl])
                    nc.sync.dma_start(out=srcT_sb[:, sl], in_=srcT[b][:, sl])

                # one-hot planes, bf16: oh[:, (t,g,j)] = (edges == t)
                # (t-major so each build op covers both halves at 4x mode)
                oh_sb = perb_pool.tile([HALF, T * 2 * N], I16, tag="oh")
                for t in range(T):
                    nc.vector.tensor_scalar(
                        out=oh_sb[:, t * 2 * N : (t + 1) * 2 * N],
                        in0=edg_sb[:],
                        scalar1=float(t),
                        scalar2=None,
                        op0=ALU.is_equal,
                    )
                # masked-edge init plane: (edges == -1) * MASK_VAL, exact fp32
                mn_sb = perb_pool.tile([HALF, 2 * N], F32, tag="mn")
                nc.vector.tensor_scalar(
                    out=mn_sb[:],
                    in0=edg_sb[:],
                    scalar1=-1.0,
                    scalar2=MASK_VAL,
                    op0=ALU.is_equal,
                    op1=ALU.mult,
                )

                for h in range(H):
                    # ---- W_t = A_t @ dstT : planes [d=64, j=256], t=0..4 ----
                    # pairs (t0,t1), (t2,t3), (t4) through [64,512] psum slots
                    w_sb = perh_pool.tile([D, T * N], F32R, tag="w")
                    for grp, (t0, nt) in enumerate(((0, 2), (2, 2), (4, 1))):
                        wp = wps_pool.tile([D, 2 * N], F32, tag="wp")
                        for k in range(nt):
                            t = t0 + k
                            nc.tensor.matmul(
                                wp[:, k * N : (k + 1) * N],
                                lhsT=at_sb[:, (h * T + t) * D : (h * T + t + 1) * D],
                                rhs=dstT_sb[:, h * N : (h + 1) * N],
                                start=True,
                                stop=True,
                            )
                        nc.scalar.copy(
                            w_sb[:, t0 * N : (t0 + nt) * N], wp[:, : nt * N]
                        )

                    for g in range(2):
                        # ---- scores planes: one psum tile [i=128, (t,j)=1280]
                        s_all = sps_pool.tile([HALF, T * N], F32, tag="s")
                        lhsT = srcT_sb[:, h * N + g * HALF : h * N + (g + 1) * HALF]
                        nc.tensor.matmul(
                            s_all[:, : 2 * N], lhsT=lhsT, rhs=w_sb[:, : 2 * N],
                            start=True, stop=True,
                        )
                        nc.tensor.matmul(
                            s_all[:, 2 * N : 4 * N], lhsT=lhsT,
                            rhs=w_sb[:, 2 * N : 4 * N], start=True, stop=True,
                        )
                        nc.tensor.matmul(
                            s_all[:, 4 * N :], lhsT=lhsT, rhs=w_sb[:, 4 * N :],
                            start=True, stop=True,
                        )

                        # ---- select: acc = mn; acc[oh_t] = S_t  (one pass) ----
                        acc = perg_pool.tile([HALF, N], F32, tag="acc")
                        # init on the (otherwise idle) gpsimd engine
                        nc.gpsimd.tensor_copy(
                            out=acc[:], in_=mn_sb[:, g * N : (g + 1) * N]
                        )
                        # out is acc broadcast over t (stride-0); masks are
                        # disjoint so the overwrite order is irrelevant.
                        acc_b = acc[:, None, :].broadcast_to((HALF, T, N))
                        mask = oh_sb[:].rearrange("p (t gj) -> p t gj", t=T)[
                            :, :, g * N : (g + 1) * N
                        ]
                        data = s_all[:].rearrange("p (t j) -> p t j", t=T)
                        nc.vector.copy_predicated(acc_b, mask, data)

                        ob = perg_pool.tile([HALF, N], F32, tag="ob")
                        nc.scalar.activation(
                            ob[:], acc[:], mybir.ActivationFunctionType.Prelu,
                            bias=0.0, scale=1.0, alpha=NEG_SLOPE,
                        )
                        nc.sync.dma_start(
                            out=out[b, h, g * HALF : (g + 1) * HALF, :], in_=ob[:]
                        )
    _split_waits(nc)
    return nc


_STATE = {}


def _get_nc():
    if "nc" not in _STATE:
        _STATE["nc"] = _build_program()
    return _STATE["nc"]


def _prep_in_maps(src_embeddings, dst_embeddings, a, edges):
    src = np.asarray(src_embeddings, dtype=np.float32)
    dst = np.asarray(dst_embeddings, dtype=np.float32)
    a = np.asarray(a, dtype=np.float32)
    # [B, N, (h d)] -> [B, d, (h i)]
    srcT = np.ascontiguousarray(
        src.reshape(B, N, H, D).transpose(0, 3, 2, 1)
    ).reshape(B, D, H * N)
    dstT = np.ascontiguousarray(
        dst.reshape(B, N, H, D).transpose(0, 3, 2, 1)
    ).reshape(B, D, H * N)
    # a: [(h d), e, t] -> at[e, (h t d)]
    at = np.ascontiguousarray(
        a.reshape(H, D, D, T).transpose(2, 0, 3, 1)
    ).reshape(D, H * T * D)
    # edges: [B, (g p), j] -> [B, p, (g j)], as bf16 (values -1..4 are exact)
    import ml_dtypes

    edgb = np.ascontiguousarray(
        edges.astype(np.float32).reshape(B, 2, HALF, N).transpose(0, 2, 1, 3)
    ).reshape(B, HALF, 2 * N).astype(ml_dtypes.bfloat16)
    in_maps = []
    for c in range(N_CORES):
        lo, hi = c * B_LOC, (c + 1) * B_LOC
        in_maps.append(
            {
                "srcT": srcT[lo:hi],
                "dstT": dstT[lo:hi],
                "at": at,
                "edgb": edgb[lo:hi],
            }
        )
    return in_maps


def run(src_embeddings, dst_embeddings, a, edges, trace=False):
    """Run on 8 cores; returns (output [B,H,N,N] f32, BassKernelResults)."""
    nc = _get_nc()
    in_maps = _prep_in_maps(src_embeddings, dst_embeddings, a, edges)
    res = bass_utils.run_bass_kernel_spmd(
        nc, in_maps, core_ids=list(range(N_CORES)), trace=trace
    )
    out = np.concatenate([res.results[c]["out"] for c in range(N_CORES)], axis=0)
    return out, res


def kernel(src_embeddings, dst_embeddings, a, edges):
    out, _ = run(src_embeddings, dst_embeddings, a, edges, trace=False)
    return out
